# revision 18
# baseline (speedup 1.0000x reference)
"""Trainium2 Bass kernel for nn_L1OutUB (L1-out upper bound contrastive loss).

Math: the reference builds a [B,B,B] tensor `inpt[a,i,j] = all_probs[i,j] +
(-20 if a==i else 0)` and logsumexps over `a`.  That logsumexp is exactly
`all_probs[i,j] + log(B-1+e^-20)`, so

    result = mean(positive) - mean(all_probs) - log1p(e^-20 / (B-1))

`sum_j all_probs[i,j]` collapses onto per-column moments of y
(S2[d] = sum_j y[j,d]^2, M1[d] = sum_j y[j,d]), the -0.5*logvar terms cancel
exactly in the difference, and the mu^2 terms of the positive branch and the
all-pairs branch cancel too, leaving a form LINEAR in mu:

    result = sum_{i,d} iv * (mu * alpha + beta) - log1p(e^-20/(B-1))
    alpha = yc/B - M1/B^2,  beta = S2/(2 B^2) - yc^2/(2B)

Sharding: rows of x across 8 cores (64 rows each); every core gets the full
(row-rotated) y so its matched rows sit at positions 0:64 and the global
column moments are unchanged by the rotation.  Host sums the 8 scalar
partials (the "all-reduce").

Layout: ONE fp16 blob [128, 1166] per core, host-packed so the device does
zero transposes and zero moment matmuls; DMA'd as two partition-halves, one
per HWDGE queue (64 x 2332B descriptors each):
  cols    0:384   xT chunks (xt[p, 64k+r] = x[r, 128k+p])
  cols  384:524   w1 packed at stride 20: window k = cols 384+20k+(0:40)
                  holds w1_mu[k] at +0:8 and w1_lv[k] at +32:40; column +8 of
                  every window is provably unoccupied (zero), so hb row 8 is
                  0 and relu(0 + bias_row8=1.0) manufactures a ones-row
  cols  524:652   w2: rows 0:8 w2_mu, row 8 = b2_mu (the hb ones-row
                  turns the K=9 L2-mu matmul into matmul+bias), rows 32:40
                  w2_lv
  col   652       b1 (rows 0:8 b1_mu, row 8 = 1.0, rows 32:40 b1_lv)
  col   653       b2_lv
  cols  654:1166  yt = rotated y^T; yc = yt[:, 0:64]
Engine plan: PE runs 6 L1 matmuls + 2 L2 matmuls; ACT does y-square /
relu / tanh / exp; DVE runs the moment reduces and the combine chain in
fp16; GPSIMD squares yc and does the final cross-partition reduce.
(ACT accum_out and DVE tensor_tensor_reduce both misbehave on this HW
path — accum_out returns garbage, TTR wedges the device — so plain
square+reduce / mul+reduce are used instead.)
"""

import numpy as np

import concourse.bacc as bacc
import concourse.tile as tile
from concourse import mybir

F32 = mybir.dt.float32
F16 = mybir.dt.float16
AF = mybir.ActivationFunctionType
ALU = mybir.AluOpType

B, X_DIM, Y_DIM, HID = 512, 768, 128, 8
N_CORES = 8
R = B // N_CORES          # rows per core = 64
XC = X_DIM // 128         # x feature chunks = 6

# blob column layout: xt | w1 | w2 | biases | yt, one partition-half per queue
W10 = 384                 # w1 stride-20 section, 140 cols
W20 = 524                 # w2 section, 128 cols
BC = 652                  # bias cols: b1(+ones row), b2_lv
YT0 = 654                 # yt section, 512 cols
A_COLS = 1166

_CACHE = {}


def _build():
    nc = bacc.Bacc("TRN2", target_bir_lowering=False, debug=False,
                   num_devices=N_CORES)

    a_d = nc.dram_tensor("a", [128, A_COLS], F16, kind="ExternalInput")
    out_d = nc.dram_tensor("out", [Y_DIM, 1], F32, kind="ExternalOutput")

    with tile.TileContext(nc) as tc:
        with (
            tc.tile_pool(name="sb", bufs=1) as sb,
            tc.tile_pool(name="ps", bufs=1, space="PSUM") as ps,
        ):
            # ---- load: one partition-half per HWDGE queue ----
            a_s = sb.tile([128, A_COLS], F16, tag="a")
            nc.sync.dma_start(out=a_s[0:64, :], in_=a_d[0:64, :])
            nc.scalar.dma_start(out=a_s[64:128, :], in_=a_d[64:128, :])
            yt = a_s[:, YT0:YT0 + B]
            yc = a_s[:, YT0:YT0 + R]

            # ---- y column moments: square on ACT, reduces on DVE ----
            ysq_s = sb.tile([128, B], F16, tag="ysq")
            nc.scalar.activation(out=ysq_s[:], in_=yt, func=AF.Square)
            m1_s = sb.tile([128, 1], F32, tag="m1")
            nc.vector.tensor_reduce(out=m1_s[:], in_=yt,
                                    axis=mybir.AxisListType.X, op=ALU.add)
            m1b_s = sb.tile([128, 1], F32, tag="m1b")
            nc.vector.tensor_scalar_mul(m1b_s[:], m1_s[:], 1.0 / (B * B))

            # ---- alpha = yc/B - M1/B^2 ; beta = S2/(2B^2) - yc^2/(2B) ----
            al_s = sb.tile([128, R], F16, tag="al")
            nc.vector.tensor_scalar(out=al_s[:], in0=yc, scalar1=1.0 / B,
                                    scalar2=m1b_s[:], op0=ALU.mult,
                                    op1=ALU.subtract)
            ycq_s = sb.tile([128, R], F32, tag="ycq")
            nc.gpsimd.tensor_mul(ycq_s[:], yc, yc)
            s2_s = sb.tile([128, 1], F32, tag="s2")
            nc.vector.tensor_reduce(out=s2_s[:], in_=ysq_s[:],
                                    axis=mybir.AxisListType.X, op=ALU.add)
            s2b_s = sb.tile([128, 1], F32, tag="s2b")
            nc.vector.tensor_scalar_mul(s2b_s[:], s2_s[:], 0.5 / (B * B))
            be_s = sb.tile([128, R], F16, tag="be")
            nc.vector.tensor_scalar(out=be_s[:], in0=ycq_s[:],
                                    scalar1=-0.5 / B, scalar2=s2b_s[:],
                                    op0=ALU.mult, op1=ALU.add)

            # ---- MLP layer 1 (both nets in one M=40 chain) ----
            hb_p = ps.tile([40, R], F32, tag="hb")
            for k in range(XC):
                nc.tensor.matmul(hb_p[:],
                                 a_s[:, W10 + 20 * k:W10 + 20 * k + 40],
                                 a_s[:, 64 * k:64 * (k + 1)],
                                 start=(k == 0), stop=(k == XC - 1))
            hb_s = sb.tile([40, R], F16, tag="hbs")
            nc.scalar.activation(out=hb_s[:], in_=hb_p[:], func=AF.Relu,
                                 bias=a_s[0:40, BC:BC + 1])

            # ---- MLP layer 2: muT (b2_mu rides the hb ones-row),
            #      lvT -> tanh -> exp(-) ----
            mu_p = ps.tile([Y_DIM, R], F32, tag="mup")
            nc.tensor.matmul(mu_p[:], a_s[0:9, W20:W20 + 128], hb_s[0:9, :],
                             start=True, stop=True)
            lv_p = ps.tile([Y_DIM, R], F32, tag="lvp")
            nc.tensor.matmul(lv_p[:], a_s[32:40, W20:W20 + 128],
                             hb_s[32:40, :], start=True, stop=True)
            lv_s = sb.tile([Y_DIM, R], F32, tag="lvs")
            nc.scalar.activation(out=lv_s[:], in_=lv_p[:], func=AF.Tanh,
                                 bias=a_s[:, BC + 1:BC + 2])
            iv_s = sb.tile([Y_DIM, R], F32, tag="ivs")
            nc.scalar.activation(out=iv_s[:], in_=lv_s[:], func=AF.Exp,
                                 scale=-1.0)

            # ---- combine: tot = sum_d iv * (mu*alpha + beta) ----
            q_s = sb.tile([Y_DIM, R], F16, tag="q")
            nc.vector.tensor_mul(q_s[:], mu_p[:], al_s[:])
            r_s = sb.tile([Y_DIM, R], F16, tag="r")
            nc.vector.tensor_add(r_s[:], q_s[:], be_s[:])
            wj_s = sb.tile([Y_DIM, R], F16, tag="wj")
            tot_s = sb.tile([Y_DIM, 1], F32, tag="tot")
            nc.vector.tensor_mul(wj_s[:], r_s[:], iv_s[:])
            nc.vector.tensor_reduce(out=tot_s[:], in_=wj_s[:],
                                    axis=mybir.AxisListType.X, op=ALU.add)
            nc.sync.dma_start(out=out_d[:], in_=tot_s[:],
                              single_packet=True)

    nc.compile()
    return nc


def _get_nc():
    if "nc" not in _CACHE:
        _CACHE["nc"] = _build()
    return _CACHE["nc"]


def _pack_weights(w1_mu, b1_mu, w2_mu, b2_mu, w1_lv, b1_lv, w2_lv, b2_lv):
    """Weights part of the crit blob: cols 0:YT0 as f32 (cast to fp16)."""
    f = np.float32
    wsec = np.zeros((128, YT0 - W10), f)
    w1m = np.asarray(w1_mu, f).reshape(XC, 128, HID)
    w1l = np.asarray(w1_lv, f).reshape(XC, 128, HID)
    for k in range(XC):
        wsec[:, 20 * k:20 * k + 8] = w1m[k]
        wsec[:, 20 * k + 32:20 * k + 40] = w1l[k]
    w2sec = wsec[:, W20 - W10:W20 - W10 + 128]
    w2sec[0:8, :] = np.asarray(w2_mu, f)
    w2sec[8, :] = np.asarray(b2_mu, f)
    w2sec[32:40, :] = np.asarray(w2_lv, f)
    bc = BC - W10
    wsec[0:8, bc] = np.asarray(b1_mu, f)
    wsec[8, bc] = 1.0
    wsec[32:40, bc] = np.asarray(b1_lv, f)
    wsec[:, bc + 1] = np.asarray(b2_lv, f)
    return wsec


def kernel(x_samples, y_samples, w1_mu, b1_mu, w2_mu, b2_mu,
           w1_lv, b1_lv, w2_lv, b2_lv, **profile_kwargs):
    from concourse import bass_utils

    f16 = np.float16
    x = np.asarray(x_samples, np.float32)
    y = np.asarray(y_samples, np.float32)
    wsec = _pack_weights(w1_mu, b1_mu, w2_mu, b2_mu,
                         w1_lv, b1_lv, w2_lv, b2_lv)
    in_maps = []
    for c in range(N_CORES):
        a = np.empty((128, A_COLS), np.float32)
        # xT chunks: a[p, 64k+r] = x[cR + r, 128k + p]
        a[:, 0:W10] = (x[c * R:(c + 1) * R]
                       .reshape(R, XC, 128).transpose(2, 1, 0)
                       .reshape(128, XC * R))
        a[:, W10:YT0] = wsec
        a[:, YT0:] = np.roll(y, -c * R, axis=0).T
        in_maps.append({"a": np.ascontiguousarray(a.astype(f16))})

    nc = _get_nc()
    res = bass_utils.run_bass_kernel_spmd(
        nc, in_maps, core_ids=list(range(N_CORES)), **profile_kwargs
    )
    total = sum(float(m["out"].sum()) for m in res.results)
    total -= np.log1p(np.exp(-20.0) / (B - 1))
    out = np.array(total, dtype=np.float32)
    if profile_kwargs:
        return out, res
    return out


# revision 20
# speedup vs baseline: 1.4398x; 1.4398x over previous
"""Trainium2 Bass kernel for nn_L1OutUB (L1-out upper bound contrastive loss).

Math: the reference builds a [B,B,B] tensor `inpt[a,i,j] = all_probs[i,j] +
(-20 if a==i else 0)` and logsumexps over `a`.  That logsumexp is exactly
`all_probs[i,j] + log(B-1+e^-20)`, so

    result = mean(positive) - mean(all_probs) - log1p(e^-20 / (B-1))

`sum_j all_probs[i,j]` collapses onto per-column moments of y
(S2[d] = sum_j y[j,d]^2, M1[d] = sum_j y[j,d]), the -0.5*logvar terms cancel
exactly in the difference, and the mu^2 terms of the positive branch and the
all-pairs branch cancel too, leaving a form LINEAR in mu:

    result = sum_{i,d} iv * (mu * alpha + beta) - log1p(e^-20/(B-1))
    alpha = yc/B - M1/B^2,  beta = S2/(2 B^2) - yc^2/(2B)

Sharding: rows of x across 8 cores (64 rows each); every core gets the full
(row-rotated) y so its matched rows sit at positions 0:64 and the global
column moments are unchanged by the rotation.  Host sums the 8 scalar
partials (the "all-reduce").

Layout: ONE fp16 blob [128, 1166] per core, host-packed so the device does
zero transposes and zero moment matmuls; DMA'd as two partition-halves, one
per HWDGE queue (64 x 2332B descriptors each):
  cols    0:384   xT chunks (xt[p, 64k+r] = x[r, 128k+p])
  cols  384:524   w1 packed at stride 20: window k = cols 384+20k+(0:40)
                  holds w1_mu[k] at +0:8 and w1_lv[k] at +32:40; column +8 of
                  every window is provably unoccupied (zero), so hb row 8 is
                  0 and relu(0 + bias_row8=1.0) manufactures a ones-row
  cols  524:652   w2: rows 0:8 w2_mu, row 8 = b2_mu (the hb ones-row
                  turns the K=9 L2-mu matmul into matmul+bias), rows 32:40
                  w2_lv
  col   652       b1 (rows 0:8 b1_mu, row 8 = 1.0, rows 32:40 b1_lv)
  col   653       b2_lv
  cols  654:1166  yt = rotated y^T; yc = yt[:, 0:64]
Engine plan: PE runs 6 L1 matmuls + 2 L2 matmuls; ACT does y-square /
relu / tanh / exp; DVE runs the moment reduces and the combine chain in
fp16; GPSIMD squares yc and does the final cross-partition reduce.
(ACT accum_out and DVE tensor_tensor_reduce both misbehave on this HW
path — accum_out returns garbage, TTR wedges the device — so plain
square+reduce / mul+reduce are used instead.)
"""

import numpy as np

import concourse.bacc as bacc
import concourse.tile as tile
from concourse import mybir

F32 = mybir.dt.float32
F16 = mybir.dt.float16
AF = mybir.ActivationFunctionType
ALU = mybir.AluOpType

B, X_DIM, Y_DIM, HID = 512, 768, 128, 8
N_CORES = 8
R = B // N_CORES          # rows per core = 64
XC = X_DIM // 128         # x feature chunks = 6

# blob column layout: xt | w1 | w2 | biases | yt, one partition-half per queue
W10 = 384                 # w1 stride-20 section, 140 cols
W20 = 524                 # w2 section, 128 cols
BC = 652                  # bias cols: b1(+ones row), b2_lv
YT0 = 654                 # yt section, 512 cols
A_COLS = 1166

_CACHE = {}


def _build():
    nc = bacc.Bacc("TRN2", target_bir_lowering=False, debug=False,
                   num_devices=N_CORES)

    a_d = nc.dram_tensor("a", [128, A_COLS], F16, kind="ExternalInput")
    out_d = nc.dram_tensor("out", [1, 1], F32, kind="ExternalOutput")

    with tile.TileContext(nc) as tc:
        with (
            tc.tile_pool(name="sb", bufs=1) as sb,
            tc.tile_pool(name="ps", bufs=1, space="PSUM") as ps,
        ):
            # ---- load: one partition-half per HWDGE queue ----
            a_s = sb.tile([128, A_COLS], F16, tag="a")
            nc.sync.dma_start(out=a_s[0:64, :], in_=a_d[0:64, :])
            nc.scalar.dma_start(out=a_s[64:128, :], in_=a_d[64:128, :])
            yt = a_s[:, YT0:YT0 + B]
            yc = a_s[:, YT0:YT0 + R]

            # ---- y column moments: square+accum on ACT (accum_out works
            # iff the main out is f32), M1 reduce on DVE ----
            ysq_s = sb.tile([128, B], F32, tag="ysq")
            s2_s = sb.tile([128, 1], F32, tag="s2")
            nc.scalar.activation(out=ysq_s[:], in_=yt, func=AF.Square,
                                 accum_out=s2_s[:])
            m1_s = sb.tile([128, 1], F32, tag="m1")
            nc.vector.tensor_reduce(out=m1_s[:], in_=yt,
                                    axis=mybir.AxisListType.X, op=ALU.add)
            m1b_s = sb.tile([128, 1], F32, tag="m1b")
            nc.vector.tensor_scalar_mul(m1b_s[:], m1_s[:], 1.0 / (B * B))

            # ---- alpha = yc/B - M1/B^2 ; beta = S2/(2B^2) - yc^2/(2B) ----
            al_s = sb.tile([128, R], F16, tag="al")
            nc.vector.tensor_scalar(out=al_s[:], in0=yc, scalar1=1.0 / B,
                                    scalar2=m1b_s[:], op0=ALU.mult,
                                    op1=ALU.subtract)
            ycq_s = sb.tile([128, R], F32, tag="ycq")
            nc.gpsimd.tensor_mul(ycq_s[:], yc, yc)
            s2b_s = sb.tile([128, 1], F32, tag="s2b")
            nc.vector.tensor_scalar_mul(s2b_s[:], s2_s[:], 0.5 / (B * B))
            be_s = sb.tile([128, R], F16, tag="be")
            nc.vector.tensor_scalar(out=be_s[:], in0=ycq_s[:],
                                    scalar1=-0.5 / B, scalar2=s2b_s[:],
                                    op0=ALU.mult, op1=ALU.add)

            # ---- MLP layer 1 (both nets in one M=40 chain) ----
            hb_p = ps.tile([40, R], F32, tag="hb")
            for k in range(XC):
                nc.tensor.matmul(hb_p[:],
                                 a_s[:, W10 + 20 * k:W10 + 20 * k + 40],
                                 a_s[:, 64 * k:64 * (k + 1)],
                                 start=(k == 0), stop=(k == XC - 1))
            hb_s = sb.tile([40, R], F16, tag="hbs")
            nc.scalar.activation(out=hb_s[:], in_=hb_p[:], func=AF.Relu,
                                 bias=a_s[0:40, BC:BC + 1])

            # ---- MLP layer 2: muT (b2_mu rides the hb ones-row),
            #      lvT -> tanh -> exp(-) ----
            mu_p = ps.tile([Y_DIM, R], F32, tag="mup")
            nc.tensor.matmul(mu_p[:], a_s[0:9, W20:W20 + 128], hb_s[0:9, :],
                             start=True, stop=True)
            lv_p = ps.tile([Y_DIM, R], F32, tag="lvp")
            nc.tensor.matmul(lv_p[:], a_s[32:40, W20:W20 + 128],
                             hb_s[32:40, :], start=True, stop=True)
            lv_s = sb.tile([Y_DIM, R], F32, tag="lvs")
            nc.scalar.activation(out=lv_s[:], in_=lv_p[:], func=AF.Tanh,
                                 bias=a_s[:, BC + 1:BC + 2])
            iv_s = sb.tile([Y_DIM, R], F32, tag="ivs")
            nc.scalar.activation(out=iv_s[:], in_=lv_s[:], func=AF.Exp,
                                 scale=-1.0)

            # ---- combine: tot = sum_d iv * (mu*alpha + beta) ----
            q_s = sb.tile([Y_DIM, R], F16, tag="q")
            nc.vector.tensor_mul(q_s[:], mu_p[:], al_s[:])
            r_s = sb.tile([Y_DIM, R], F16, tag="r")
            nc.vector.tensor_add(r_s[:], q_s[:], be_s[:])
            wj_s = sb.tile([Y_DIM, R], F16, tag="wj")
            tot_s = sb.tile([Y_DIM, 1], F32, tag="tot")
            nc.vector.tensor_mul(wj_s[:], r_s[:], iv_s[:])
            nc.vector.tensor_reduce(out=tot_s[:], in_=wj_s[:],
                                    axis=mybir.AxisListType.X, op=ALU.add)
            res_s = sb.tile([1, 1], F32, tag="ress")
            nc.gpsimd.tensor_reduce(out=res_s[:], in_=tot_s[:],
                                    axis=mybir.AxisListType.C, op=ALU.add)
            nc.sync.dma_start(out=out_d[:], in_=res_s[:],
                              single_packet=True)

    nc.compile()
    return nc


def _get_nc():
    if "nc" not in _CACHE:
        _CACHE["nc"] = _build()
    return _CACHE["nc"]


def _pack_weights(w1_mu, b1_mu, w2_mu, b2_mu, w1_lv, b1_lv, w2_lv, b2_lv):
    """Weights part of the crit blob: cols 0:YT0 as f32 (cast to fp16)."""
    f = np.float32
    wsec = np.zeros((128, YT0 - W10), f)
    w1m = np.asarray(w1_mu, f).reshape(XC, 128, HID)
    w1l = np.asarray(w1_lv, f).reshape(XC, 128, HID)
    for k in range(XC):
        wsec[:, 20 * k:20 * k + 8] = w1m[k]
        wsec[:, 20 * k + 32:20 * k + 40] = w1l[k]
    w2sec = wsec[:, W20 - W10:W20 - W10 + 128]
    w2sec[0:8, :] = np.asarray(w2_mu, f)
    w2sec[8, :] = np.asarray(b2_mu, f)
    w2sec[32:40, :] = np.asarray(w2_lv, f)
    bc = BC - W10
    wsec[0:8, bc] = np.asarray(b1_mu, f)
    wsec[8, bc] = 1.0
    wsec[32:40, bc] = np.asarray(b1_lv, f)
    wsec[:, bc + 1] = np.asarray(b2_lv, f)
    return wsec


def kernel(x_samples, y_samples, w1_mu, b1_mu, w2_mu, b2_mu,
           w1_lv, b1_lv, w2_lv, b2_lv, **profile_kwargs):
    from concourse import bass_utils

    f16 = np.float16
    x = np.asarray(x_samples, np.float32)
    y = np.asarray(y_samples, np.float32)
    wsec = _pack_weights(w1_mu, b1_mu, w2_mu, b2_mu,
                         w1_lv, b1_lv, w2_lv, b2_lv)
    in_maps = []
    for c in range(N_CORES):
        a = np.empty((128, A_COLS), np.float32)
        # xT chunks: a[p, 64k+r] = x[cR + r, 128k + p]
        a[:, 0:W10] = (x[c * R:(c + 1) * R]
                       .reshape(R, XC, 128).transpose(2, 1, 0)
                       .reshape(128, XC * R))
        a[:, W10:YT0] = wsec
        a[:, YT0:] = np.roll(y, -c * R, axis=0).T
        in_maps.append({"a": np.ascontiguousarray(a.astype(f16))})

    nc = _get_nc()
    res = bass_utils.run_bass_kernel_spmd(
        nc, in_maps, core_ids=list(range(N_CORES)), **profile_kwargs
    )
    total = sum(float(m["out"][0, 0]) for m in res.results)
    total -= np.log1p(np.exp(-20.0) / (B - 1))
    out = np.array(total, dtype=np.float32)
    if profile_kwargs:
        return out, res
    return out


# revision 21
# speedup vs baseline: 1.5098x; 1.0487x over previous
"""Trainium2 Bass kernel for nn_L1OutUB (L1-out upper bound contrastive loss).

Math: the reference builds a [B,B,B] tensor `inpt[a,i,j] = all_probs[i,j] +
(-20 if a==i else 0)` and logsumexps over `a`.  That logsumexp is exactly
`all_probs[i,j] + log(B-1+e^-20)`, so

    result = mean(positive) - mean(all_probs) - log1p(e^-20 / (B-1))

`sum_j all_probs[i,j]` collapses onto per-column moments of y
(S2[d] = sum_j y[j,d]^2, M1[d] = sum_j y[j,d]), the -0.5*logvar terms cancel
exactly in the difference, and the mu^2 terms of the positive branch and the
all-pairs branch cancel too, leaving a form LINEAR in mu:

    result = sum_{i,d} iv * (mu * alpha + beta) - log1p(e^-20/(B-1))
    alpha = yc/B - M1/B^2,  beta = S2/(2 B^2) - yc^2/(2B)

Sharding: rows of x across 8 cores (64 rows each); every core gets the full
(row-rotated) y so its matched rows sit at positions 0:64 and the global
column moments are unchanged by the rotation.  Host sums the 8 scalar
partials (the "all-reduce").

Layout: ONE fp16 blob [128, 1166] per core, host-packed so the device does
zero transposes and zero moment matmuls; DMA'd as two partition-halves, one
per HWDGE queue (64 x 2332B descriptors each):
  cols    0:384   xT chunks (xt[p, 64k+r] = x[r, 128k+p])
  cols  384:524   w1 packed at stride 20: window k = cols 384+20k+(0:40)
                  holds w1_mu[k] at +0:8 and w1_lv[k] at +32:40; column +8 of
                  every window is provably unoccupied (zero), so hb row 8 is
                  0 and relu(0 + bias_row8=1.0) manufactures a ones-row
  cols  524:652   w2: rows 0:8 w2_mu, row 8 = b2_mu (the hb ones-row
                  turns the K=9 L2-mu matmul into matmul+bias), rows 32:40
                  w2_lv
  col   652       b1 (rows 0:8 b1_mu, row 8 = 1.0, rows 32:40 b1_lv)
  col   653       b2_lv
  cols  654:1166  yt = rotated y^T; yc = yt[:, 0:64]
Engine plan: PE runs 6 L1 matmuls + 2 L2 matmuls; ACT does y-square /
relu / tanh / exp; DVE runs the moment reduces and the combine chain in
fp16; GPSIMD squares yc and does the final cross-partition reduce.
(ACT accum_out and DVE tensor_tensor_reduce both misbehave on this HW
path — accum_out returns garbage, TTR wedges the device — so plain
square+reduce / mul+reduce are used instead.)
"""

import numpy as np

import concourse.bacc as bacc
import concourse.tile as tile
from concourse import mybir

F32 = mybir.dt.float32
F16 = mybir.dt.float16
AF = mybir.ActivationFunctionType
ALU = mybir.AluOpType

B, X_DIM, Y_DIM, HID = 512, 768, 128, 8
N_CORES = 8
R = B // N_CORES          # rows per core = 64
XC = X_DIM // 128         # x feature chunks = 6

# blob column layout: xt | w1 | w2 | biases | yt, one partition-half per queue
W10 = 384                 # w1 stride-20 section, 140 cols
W20 = 524                 # w2 section, 128 cols
BC = 652                  # bias cols: b1(+ones row), b2_lv
YT0 = 654                 # yt section, 512 cols
A_COLS = 1166

_CACHE = {}


def _build():
    nc = bacc.Bacc("TRN2", target_bir_lowering=False, debug=False,
                   num_devices=N_CORES)

    a_d = nc.dram_tensor("a", [128, A_COLS], F16, kind="ExternalInput")
    out_d = nc.dram_tensor("out", [1, 1], F32, kind="ExternalOutput")

    with tile.TileContext(nc) as tc:
        with (
            tc.tile_pool(name="sb", bufs=1) as sb,
            tc.tile_pool(name="ps", bufs=1, space="PSUM") as ps,
        ):
            # ---- load: one partition-half per HWDGE queue ----
            a_s = sb.tile([128, A_COLS], F16, tag="a")
            nc.sync.dma_start(out=a_s[0:64, :], in_=a_d[0:64, :])
            nc.scalar.dma_start(out=a_s[64:128, :], in_=a_d[64:128, :])
            yt = a_s[:, YT0:YT0 + B]
            yc = a_s[:, YT0:YT0 + R]

            # ---- y column moments: square+accum on ACT (accum_out works
            # iff the main out is f32), M1 reduce on DVE ----
            ysq_s = sb.tile([128, B], F32, tag="ysq")
            s2_s = sb.tile([128, 1], F32, tag="s2")
            nc.scalar.activation(out=ysq_s[:], in_=yt, func=AF.Square,
                                 accum_out=s2_s[:])
            m1_s = sb.tile([128, 1], F32, tag="m1")
            nc.vector.tensor_reduce(out=m1_s[:], in_=yt,
                                    axis=mybir.AxisListType.X, op=ALU.add)
            m1b_s = sb.tile([128, 1], F32, tag="m1b")
            nc.vector.tensor_scalar_mul(m1b_s[:], m1_s[:], 1.0 / (B * B))

            # ---- alpha = yc/B - M1/B^2 ; beta = S2/(2B^2) - yc^2/(2B) ----
            al_s = sb.tile([128, R], F16, tag="al")
            nc.vector.tensor_scalar(out=al_s[:], in0=yc, scalar1=1.0 / B,
                                    scalar2=m1b_s[:], op0=ALU.mult,
                                    op1=ALU.subtract)
            ycq_s = sb.tile([128, R], F32, tag="ycq")
            nc.gpsimd.tensor_mul(ycq_s[:], yc, yc)
            s2b_s = sb.tile([128, 1], F32, tag="s2b")
            nc.vector.tensor_scalar_mul(s2b_s[:], s2_s[:], 0.5 / (B * B))
            be_s = sb.tile([128, R], F16, tag="be")
            nc.vector.tensor_scalar(out=be_s[:], in0=ycq_s[:],
                                    scalar1=-0.5 / B, scalar2=s2b_s[:],
                                    op0=ALU.mult, op1=ALU.add)

            # ---- MLP layer 1 (both nets in one M=40 chain) ----
            hb_p = ps.tile([40, R], F32, tag="hb")
            for k in range(XC):
                nc.tensor.matmul(hb_p[:],
                                 a_s[:, W10 + 20 * k:W10 + 20 * k + 40],
                                 a_s[:, 64 * k:64 * (k + 1)],
                                 start=(k == 0), stop=(k == XC - 1))
            hb_s = sb.tile([40, R], F16, tag="hbs")
            nc.scalar.activation(out=hb_s[:], in_=hb_p[:], func=AF.Relu,
                                 bias=a_s[0:40, BC:BC + 1])

            # ---- MLP layer 2: muT (b2_mu rides the hb ones-row),
            #      lvT -> tanh -> exp(-) ----
            mu_p = ps.tile([Y_DIM, R], F32, tag="mup")
            nc.tensor.matmul(mu_p[:], a_s[0:9, W20:W20 + 128], hb_s[0:9, :],
                             start=True, stop=True)
            lv_p = ps.tile([Y_DIM, R], F32, tag="lvp")
            nc.tensor.matmul(lv_p[:], a_s[32:40, W20:W20 + 128],
                             hb_s[32:40, :], start=True, stop=True)
            lv_s = sb.tile([Y_DIM, R], F32, tag="lvs")
            nc.scalar.activation(out=lv_s[:], in_=lv_p[:], func=AF.Tanh,
                                 bias=a_s[:, BC + 1:BC + 2])
            iv_s = sb.tile([Y_DIM, R], F32, tag="ivs")
            nc.scalar.activation(out=iv_s[:], in_=lv_s[:], func=AF.Exp,
                                 scale=-1.0)

            # ---- combine: tot = sum_d iv * (mu*alpha + beta) ----
            q_s = sb.tile([Y_DIM, R], F16, tag="q")
            nc.vector.tensor_mul(q_s[:], mu_p[:], al_s[:])
            r_s = sb.tile([Y_DIM, R], F16, tag="r")
            nc.vector.tensor_add(r_s[:], q_s[:], be_s[:])
            wj_s = sb.tile([Y_DIM, R], F16, tag="wj")
            tot_s = sb.tile([Y_DIM, 1], F32, tag="tot")
            nc.vector.tensor_mul(wj_s[:], r_s[:], iv_s[:])
            nc.vector.tensor_reduce(out=tot_s[:], in_=wj_s[:],
                                    axis=mybir.AxisListType.X, op=ALU.add)
            res_s = sb.tile([1, 1], F32, tag="ress")
            nc.gpsimd.tensor_reduce(out=res_s[:], in_=tot_s[:],
                                    axis=mybir.AxisListType.C, op=ALU.add)
            nc.sync.dma_start(out=out_d[:], in_=res_s[:],
                              single_packet=True)

    # Hoist the two input DMAs from the tile block to the main block, right
    # after their engines' register-init, so the ~1.2us of DMA issue + ring
    # startup overlaps the const-memset + entry-barrier prelude instead of
    # starting after it.  Their completion-semaphore updates (and the
    # consumers' waits inside the tile block) are untouched.
    fn = nc.m.functions[0]
    main_blk, tile_blk = fn.blocks[0], fn.blocks[1]
    dmas = [ins for ins in list(tile_blk.instructions)[:4]
            if type(ins).__name__ == "InstDMACopy"]
    assert len(dmas) == 2, [type(i).__name__ for i in tile_blk.instructions[:4]]
    for ins in dmas:
        tile_blk.instructions.remove(ins)
        idx = max(i for i, m in enumerate(main_blk.instructions)
                  if getattr(m, "engine", None) == ins.engine
                  and type(m).__name__ in ("InstRegisterMove", "InstTPBBaseLd"))
        main_blk.instructions.insert(idx + 1, ins)

    nc.compile()
    return nc


def _get_nc():
    if "nc" not in _CACHE:
        _CACHE["nc"] = _build()
    return _CACHE["nc"]


def _pack_weights(w1_mu, b1_mu, w2_mu, b2_mu, w1_lv, b1_lv, w2_lv, b2_lv):
    """Weights part of the crit blob: cols 0:YT0 as f32 (cast to fp16)."""
    f = np.float32
    wsec = np.zeros((128, YT0 - W10), f)
    w1m = np.asarray(w1_mu, f).reshape(XC, 128, HID)
    w1l = np.asarray(w1_lv, f).reshape(XC, 128, HID)
    for k in range(XC):
        wsec[:, 20 * k:20 * k + 8] = w1m[k]
        wsec[:, 20 * k + 32:20 * k + 40] = w1l[k]
    w2sec = wsec[:, W20 - W10:W20 - W10 + 128]
    w2sec[0:8, :] = np.asarray(w2_mu, f)
    w2sec[8, :] = np.asarray(b2_mu, f)
    w2sec[32:40, :] = np.asarray(w2_lv, f)
    bc = BC - W10
    wsec[0:8, bc] = np.asarray(b1_mu, f)
    wsec[8, bc] = 1.0
    wsec[32:40, bc] = np.asarray(b1_lv, f)
    wsec[:, bc + 1] = np.asarray(b2_lv, f)
    return wsec


def kernel(x_samples, y_samples, w1_mu, b1_mu, w2_mu, b2_mu,
           w1_lv, b1_lv, w2_lv, b2_lv, **profile_kwargs):
    from concourse import bass_utils

    f16 = np.float16
    x = np.asarray(x_samples, np.float32)
    y = np.asarray(y_samples, np.float32)
    wsec = _pack_weights(w1_mu, b1_mu, w2_mu, b2_mu,
                         w1_lv, b1_lv, w2_lv, b2_lv)
    in_maps = []
    for c in range(N_CORES):
        a = np.empty((128, A_COLS), np.float32)
        # xT chunks: a[p, 64k+r] = x[cR + r, 128k + p]
        a[:, 0:W10] = (x[c * R:(c + 1) * R]
                       .reshape(R, XC, 128).transpose(2, 1, 0)
                       .reshape(128, XC * R))
        a[:, W10:YT0] = wsec
        a[:, YT0:] = np.roll(y, -c * R, axis=0).T
        in_maps.append({"a": np.ascontiguousarray(a.astype(f16))})

    nc = _get_nc()
    res = bass_utils.run_bass_kernel_spmd(
        nc, in_maps, core_ids=list(range(N_CORES)), **profile_kwargs
    )
    total = sum(float(m["out"][0, 0]) for m in res.results)
    total -= np.log1p(np.exp(-20.0) / (B - 1))
    out = np.array(total, dtype=np.float32)
    if profile_kwargs:
        return out, res
    return out
